# revision 27
# baseline (speedup 1.0000x reference)
"""MoE transformer layer (soft-routed) on 8 TRN2 NeuronCores.

Strategy: pure batch-data-parallel (B == n_cores == 8); each core runs the
full layer for one batch element, no collectives. All heavy matmuls run in
fp8e4 DoubleRow mode (2 fp8 weights per PE cell -> 0.5 cycles/row, 2x the
fp32r/bf16 rate). Accumulation stays fp32 in PSUM. The layer-scale structure
(gamma = 1e-5 on both residual branches) leaves enormous numerical headroom,
so fp8/bf16 quantization of weights/activations is far inside tolerance.

Engine balance (per the instruction cost model):
  - PE: all projections fp8 DoubleRow, 2 moving matmuls per LDWEIGHTS pair.
  - ACT: softmax exp of 3 in 4 heads, silu, PSUM->SBUF wide copies.
  - DVE: epilogue scalar_tensor_tensor ops (2x all-SBUF mode where possible,
    bf16 rope chain for the 2x 16-bit mode), exp of 1 in 4 heads via pow.
  - The whole routed+shared FFN down-projection accumulates in PSUM across
    all 96 h-tiles (routing weights folded into the gate epilogue), so the
    down epilogue is one scalar_tensor_tensor per (dt, seq-half, round).

Scale bookkeeping (power-of-2, folded into existing epilogues at zero cost):
  weights * 2^11; xn, xn2, attn * 16 (via a 16.0-row in the rstd/softmax
  broadcast matmuls); v * 32 (softmax-denom column is 32 too -> ratio exact);
  hidden * 8 * routing_weight (via the per-group dscale column). Dequants live
  in the cos/sin tables (q also carries 1/sqrt(dh)), gamma columns, and the
  silu activation scale.
"""
import sys, os

for _p in ("/opt/trn_rl_repo", "/root/.axon_site/_ro/trn_rl_repo"):
    if os.path.isdir(_p) and _p not in sys.path:
        sys.path.insert(0, _p)

import numpy as np
import ml_dtypes
import concourse.bacc as bacc
import concourse.mybir as mybir
from concourse import bass_utils
from concourse.tile import TileContext

f32 = mybir.dt.float32
f32r = mybir.dt.float32r
bf16 = mybir.dt.bfloat16
f8 = mybir.dt.float8e4
AF = mybir.ActivationFunctionType
OP = mybir.AluOpType
PM = mybir.MatmulPerfMode
NPF8 = ml_dtypes.float8_e4m3   # TRN FP8_EXP4: max +-240, has inf
NPBF = ml_dtypes.bfloat16

FULL = dict(D=1024, S=1024, NH=16, SH=4096, RH=1024, NE=8, NCORES=8)
EPS = 1e-5
ROPE_SCALES = (1.0, 1.0, 1.0, 0.2)
ROPE_BASE = 10000.0

SW = 11        # weight scale exponent (all weight tensors)
SX = 4         # xn / xn2 / attn activation scale exponent (16)
SV = 5         # v scale exponent (32)
SHID = 3       # ffn hidden scale exponent (8)


def _to_f8(a):
    return np.clip(a, -240.0, 240.0).astype(NPF8)


def _pack_w(WT, mw, scale=float(2 ** SW)):
    """WT [in_f, out_f] fp32 -> fp8 packed [128, nb*kd*mw] with per-partition
    layout [b, k, m]: block b covers output cols [b*mw,(b+1)*mw), k indexes
    the 128-row contraction tiles. A block DMA is 128 lines of kd*mw bytes."""
    in_f, out_f = WT.shape
    kd, nb = in_f // 128, out_f // mw
    a = (np.asarray(WT, np.float64) * scale).reshape(kd, 128, nb, mw)
    a = a.transpose(1, 2, 0, 3).reshape(128, nb * kd * mw)
    return _to_f8(a)


# ---------------------------------------------------------------- host prep

def host_prep(cfg, src, centroids, routing_weights, qkv_w, qkv_b, out_w, out_b,
              norm1_w, norm2_w, gamma_1, gamma_2,
              sh_wg, sh_bg, sh_wu, sh_bu, sh_wd, sh_bd,
              r_wg, r_bg, r_wu, r_bu, r_wd, r_bd):
    """Returns list of per-core input dicts (one batch element per core)."""
    D, S, NH, SH, RH, NE = cfg["D"], cfg["S"], cfg["NH"], cfg["SH"], cfg["RH"], cfg["NE"]
    DH = D // NH
    KD = D // 128
    B = src.shape[0]
    c = lambda a: np.ascontiguousarray(a, dtype=np.float32)
    cb = lambda a: np.ascontiguousarray(np.asarray(a, np.float32).astype(NPBF))
    scale = 1.0 / np.sqrt(DH)

    assert not np.any(qkv_b) and not np.any(out_b), "nonzero qkv/out bias unsupported"
    assert not np.any(sh_bd) and not np.any(r_bd), "nonzero down-proj bias unsupported"
    assert not (np.any(sh_bg) or np.any(sh_bu) or np.any(r_bg) or np.any(r_bu)), \
        "nonzero gate/up bias unsupported"

    qkvT = qkv_w.T * norm1_w[:, None]          # [D, 3D], no score-scale fold
    shwgT = sh_wg.T * norm2_w[:, None]
    shwuT = sh_wu.T * norm2_w[:, None]

    # down: all 96 h-tiles (shared 0..31, expert e at 32+8e) in one matrix,
    # packed per (dt, round r of 48 h-tiles): [128, 48i, 128j]
    big_wd = np.concatenate([np.asarray(sh_wd).T]
                            + [np.asarray(r_wd[e]).T for e in range(NE)], axis=0)
    wd_all = _to_f8((big_wd * 2.0 ** SW).reshape(2, 48, 128, 8, 128)
                    .transpose(2, 3, 0, 1, 4).reshape(128, -1))

    shared = {
        "qk_w": _pack_w(qkvT[:, : 2 * D], 128),          # blocks: q hp 0..7, k 8..15
        "v_w": _pack_w(qkvT[:, 2 * D:], 512),            # 2 blocks of 8 heads
        "out_w8": _pack_w(out_w.T, 128),                 # 8 blocks (et)
        "g1_col": c((np.asarray(gamma_1) * 2.0 ** (-(SX + SW))).reshape(KD, 128).T),
        "g2_col": c((np.asarray(gamma_2) * 2.0 ** (-(SHID + SW))).reshape(KD, 128).T),
        "sh_wg8": _pack_w(shwgT, 128),                   # 32 blocks (ht)
        "sh_wu8": _pack_w(shwuT, 128),
        "wd_all": wd_all,
        "r_wg8": np.ascontiguousarray(
            np.stack([_pack_w(r_wg[e].T * norm2_w[:, None], 128) for e in range(NE)])
            .reshape(NE, 128, -1).transpose(1, 0, 2).reshape(128, -1)),
        "r_wu8": np.ascontiguousarray(
            np.stack([_pack_w(r_wu[e].T * norm2_w[:, None], 128) for e in range(NE)])
            .reshape(NE, 128, -1).transpose(1, 0, 2).reshape(128, -1)),
    }
    perm = np.zeros((128, 128), np.float32)
    perm[np.arange(128) ^ 8, np.arange(128)] = 1.0
    shared["perm"] = cb(perm)

    # rope tables: row p handles head-local feature dh = p % DH
    p = np.arange(128)
    dh = p % DH
    d_axis = DH // 4
    half = d_axis // 2
    a_idx = dh // d_axis
    j = dh % d_axis
    f = j % half
    sign = np.where(j < half, -1.0, 1.0).astype(np.float32)
    inv_freq = (1.0 / (ROPE_BASE ** (np.arange(half) / half))).astype(np.float32)

    dq = scale * 2.0 ** (-(SX + SW))   # q dequant incl. 1/sqrt(dh)
    dk = 2.0 ** (-(SX + SW))
    in_maps = []
    for b in range(B):
        m = dict(shared)
        m["srcT"] = c(np.asarray(src[b]).T)
        pos = np.asarray(centroids[b])[:, a_idx] * np.asarray(ROPE_SCALES, np.float32)[a_idx]
        ang = (pos * inv_freq[f][None, :]).T.astype(np.float32)        # [128, S]
        cosv, sinv = np.cos(ang), sign[:, None] * np.sin(ang)
        m["cosqT"], m["sinqT"] = cb(cosv * dq), cb(sinv * dq)
        m["coskT"], m["sinkT"] = cb(cosv * dk), cb(sinv * dk)
        rw = np.asarray(routing_weights[b], np.float32)
        ds = np.empty((12,), np.float32)
        ds[:4] = 2.0 ** (SHID - SX - SW)
        ds[4:] = rw * 2.0 ** (SHID - SX - SW)
        m["dscale"] = c(np.broadcast_to(ds, (128, 12)))
        in_maps.append(m)
    return in_maps


# ---------------------------------------------------------------- device build

def build_nc(cfg, reps=1):
    STOP = os.environ.get("STOP_AFTER", "")
    SKIP_ATT = bool(os.environ.get("SKIP_ATT"))
    HP_LIM = int(os.environ.get("HP_LIM", "8"))
    SKIP_FFN = bool(os.environ.get("SKIP_FFN"))
    D, S, NH, SH, RH, NE = cfg["D"], cfg["S"], cfg["NH"], cfg["SH"], cfg["RH"], cfg["NE"]
    DH = D // NH
    assert DH == 64 and D % 128 == 0 and S % 128 == 0 and SH % 128 == 0 and RH % 128 == 0
    KD, ST, SHT, RHT = D // 128, S // 128, SH // 128, RH // 128
    CH = min(512, S)
    NCH = S // CH
    HP = NH // 2          # head pairs
    VS = 2 * DH           # v stride per head (64 data + 64 denom cols of 2.0)
    KP = KD // 2          # contraction k-tile pairs
    NHT = SHT + NE * RHT  # 96 total h-tiles
    ROUND = NHT // 2      # h-tiles per down round

    nc = bacc.Bacc("TRN2", target_bir_lowering=False, debug=False)

    def din(name, shape, dt=f32):
        return nc.dram_tensor(name, list(shape), dt, kind="ExternalInput")

    srcT_d = din("srcT", (D, S))
    cosq_d = din("cosqT", (128, S), bf16)
    sinq_d = din("sinqT", (128, S), bf16)
    cosk_d = din("coskT", (128, S), bf16)
    sink_d = din("sinkT", (128, S), bf16)
    perm_d = din("perm", (128, 128), bf16)
    qk_d = din("qk_w", (128, 16 * KD * 128), f8)
    v_d = din("v_w", (128, 2 * KD * 512), f8)
    outw_d = din("out_w8", (128, KD * KD * 128), f8)
    g1_d = din("g1_col", (128, KD))
    g2_d = din("g2_col", (128, KD))
    shwg_d = din("sh_wg8", (128, SHT * KD * 128), f8)
    shwu_d = din("sh_wu8", (128, SHT * KD * 128), f8)
    wd_d = din("wd_all", (128, KD * 2 * ROUND * 128), f8)
    rwg_d = din("r_wg8", (128, NE * RHT * KD * 128), f8)
    rwu_d = din("r_wu8", (128, NE * RHT * KD * 128), f8)
    dsc_d = din("dscale", (128, 12))
    outT_d = nc.dram_tensor("outT", [D, S], f32, kind="ExternalOutput")

    def load_blk(pool, dram_ap, nb, b, width, tag, bufs, name):
        """Load packed weight block b: [128, width] contiguous, fp8."""
        t = pool.tile([128, width], f8, name=name, tag=tag, bufs=bufs)
        src = dram_ap.rearrange("p (b x) -> p b x", b=nb)[:, b, :]
        nc.sync.dma_start(t[:], src)
        return t

    with TileContext(nc) as tc:
      for rep_i in range(reps):
          cpool = tc.alloc_tile_pool(name=f"const{rep_i}", bufs=1)

          onesf = cpool.tile([128, 32], f32, name="onesf", tag="onesf")
          nc.vector.memset(onesf[:], 1.0)
          c2f8 = cpool.tile([128, 64], f8, name="c2f8", tag="c2f8")
          nc.vector.memset(c2f8[:], 2.0)
          row16f = cpool.tile([1, 128], f32, name="row16f", tag="row16f")
          nc.vector.memset(row16f[:], 16.0)
          row16 = cpool.tile([1, 128], f32r, name="row16", tag="row16")
          nc.vector.tensor_copy(row16[:], row16f[:])
          ones_col = cpool.tile([128, 1], f32r, name="ones_col", tag="ones_col")
          nc.vector.tensor_copy(ones_col[:], onesf[:, 0:1])
          eps1 = cpool.tile([1, 1], f32, name="eps1", tag="eps1")
          nc.vector.memset(eps1[:], EPS)
          e_t = cpool.tile([128, 2 * CH], f32, name="e_t", tag="e_t")
          nc.vector.memset(e_t[:], float(np.e))
          permt = cpool.tile([128, 128], bf16, name="permt", tag="permt")
          nc.sync.dma_start(permt[:], perm_d.ap())
          g1c = cpool.tile([128, KD], f32, name="g1c", tag="g1c")
          nc.sync.dma_start(g1c[:], g1_d.ap())
          g2c = cpool.tile([128, KD], f32, name="g2c", tag="g2c")
          nc.sync.dma_start(g2c[:], g2_d.ap())
          dsc = cpool.tile([128, 12], f32, name="dsc", tag="dsc")
          nc.sync.dma_start(dsc[:], dsc_d.ap())

          x1pool = tc.alloc_tile_pool(name=f"x1p{rep_i}", bufs=1)
          x1T = [x1pool.tile([128, S], f32, name=f"x1T{kt}", tag=f"x1T{kt}")
                 for kt in range(KD)]

          # attn output in fp8 [k, s] layout (lives until phase C)
          atP = tc.alloc_tile_pool(name=f"atP{rep_i}", bufs=1)
          at_f8 = atP.tile([128, KD * S], f8, name="at_f8", tag="at_f8")
          at3 = at_f8[:].rearrange("p (k s) -> p k s", k=KD)

          # rope tables (bf16), released after attention
          ropeP = tc.alloc_tile_pool(name=f"ropeP{rep_i}", bufs=1)
          rope_t = {}
          for nm, d in (("cosq", cosq_d), ("sinq", sinq_d),
                        ("cosk", cosk_d), ("sink", sink_d)):
              t = ropeP.tile([128, S], bf16, name=nm, tag=nm)
              nc.sync.dma_start(t[:], d.ap())
              rope_t[nm] = t

          # ---------------- phase A: rms norm 1 -> xn_f8 (= src * rstd * 16)
          xnP = tc.alloc_tile_pool(name=f"xnP{rep_i}", bufs=1)
          xn_f8 = xnP.tile([128, KD * S], f8, name="xn_f8", tag="xn_f8")
          xn3 = xn_f8[:].rearrange("p (k s) -> p k s", k=KD)

          srcA = tc.alloc_tile_pool(name=f"srcA{rep_i}", bufs=1)
          psA = tc.alloc_tile_pool(name=f"psA{rep_i}", bufs=1, space="PSUM")
          sqA = tc.alloc_tile_pool(name=f"sqA{rep_i}", bufs=1)
          srcT = []
          for kt in range(KD):
              t = srcA.tile([128, S], f32, name=f"srcT{kt}", tag=f"srcT{kt}")
              nc.sync.dma_start(t[:], srcT_d.ap()[kt * 128:(kt + 1) * 128, :])
              srcT.append(t)
          for c in range(NCH):
              cs = slice(c * CH, (c + 1) * CH)
              vrow_ps = psA.tile([1, CH], f32, name="vrow_ps", tag="vrow", bufs=2)
              for kt in range(KD):
                  sq = sqA.tile([128, CH], f32r, name="sq", tag="sq", bufs=3)
                  nc.vector.scalar_tensor_tensor(sq[:], srcT[kt][:, cs], 1.0,
                                                 srcT[kt][:, cs],
                                                 op0=OP.mult, op1=OP.mult)
                  nc.tensor.matmul(vrow_ps[:], ones_col[:], sq[:],
                                   start=(kt == 0), stop=(kt == KD - 1))
              srr = sqA.tile([1, 2 * CH], f32r, name="srr", tag="srr", bufs=2)
              srow = srr[0:1, 0:CH]
              rrow = srr[0:1, CH:2 * CH]
              nc.scalar.activation(srow, vrow_ps[:], AF.Sqrt,
                                   bias=eps1[:1, 0:1], scale=1.0 / D)
              with nc.allow_low_precision(reason="rstd fp32r rounding ok"):
                  nc.vector.reciprocal(rrow, srow)
              bc = psA.tile([128, CH], f32, name="bcA", tag="bcA", bufs=2)
              nc.tensor.matmul(bc[:], row16[:1, 0:128], rrow, start=True, stop=True)
              bcs = sqA.tile([128, CH], f32, name="bcsA", tag="bcsA", bufs=2)
              nc.scalar.activation(bcs[:], bc[:], AF.Copy)
              for kt in range(KD):
                  nc.vector.scalar_tensor_tensor(xn3[:, kt, cs], srcT[kt][:, cs],
                                                 1.0, bcs[:],
                                                 op0=OP.mult, op1=OP.mult)
          sqA.release()
          psA.release()
          srcA.release()
          if STOP == "A":
              nc.sync.dma_start(outT_d.ap()[0:128, :], xn_f8[:, 0:4096].bitcast(f32))
              xnP.release(); ropeP.release(); atP.release(); x1pool.release(); cpool.release()
              nc.compile(); return nc

          # ---------------- phase B: attention --------------------------
          if SKIP_ATT or HP_LIM < 8:
              nc.vector.memset(at_f8[:], 0.125)
          wB = tc.alloc_tile_pool(name=f"wB{rep_i}", bufs=1)
          qkB = tc.alloc_tile_pool(name=f"qkB{rep_i}", bufs=1)
          vB = tc.alloc_tile_pool(name=f"vB{rep_i}", bufs=1)
          psB = tc.alloc_tile_pool(name=f"psB{rep_i}", bufs=1, space="PSUM")

          # ---- v in [s-pair, 2, heads*VS] fp8 layout, denom col = 32 ----
          v2 = [vB.tile([128, 2 * NH * VS], f8, name=f"v{pr}", tag=f"v{pr}")
                for pr in range(ST // 2)]
          for pr in range(ST // 2 if not SKIP_ATT else 0):
              oc = v2[pr][:].rearrange("p (k h c) -> p k h c", k=2, c=VS)[:, :, :, DH:2 * DH]
              nc.vector.tensor_copy(
                  oc, c2f8[:, None, None, :].to_broadcast((128, 2, NH, DH)))
          wv = [load_blk(wB, v_d.ap(), 2, vb, KD * 512, tag=f"wv{vb}", bufs=1,
                         name=f"wv{vb}") for vb in range(2)] if not SKIP_ATT else []
          wv3 = [w[:].rearrange("p (k m) -> p k m", k=KD) for w in wv]
          for st in range(ST if not SKIP_ATT else 0):
              pv2 = psB.tile([128, 2 * CH], f32, name="pv2", tag="W", bufs=2)
              for kp in range(KP):
                  lhsT = xn3[:, 2 * kp:2 * kp + 2, st * 128:(st + 1) * 128]
                  for vb in range(2):
                      nc.tensor.matmul(pv2[:, vb * CH:(vb + 1) * CH], lhsT,
                                       wv3[vb][:, 2 * kp:2 * kp + 2, :],
                                       start=(kp == 0), stop=(kp == KP - 1),
                                       perf_mode=PM.DoubleRow)
              dst = v2[st // 2][:].rearrange("p (k h c) -> p k h c", k=2, c=VS)[
                  :, st % 2, :, 0:DH]
              nc.scalar.activation(dst, pv2[:].rearrange("p (h c) -> p h c", c=DH),
                                   AF.Copy, scale=2.0 ** (SV - SX - SW))

          if STOP == "BV":
              nc.sync.dma_start(outT_d.ap()[0:128, :], v2[0][:, 0:2048].bitcast(f32)[:, 0:1024])
              psB.release(); vB.release(); qkB.release(); wB.release(); xnP.release(); ropeP.release()
              atP.release(); x1pool.release(); cpool.release()
              nc.compile(); return nc

          # ---- per head pair: qk proj, rope, scores, av ------------
          for hp in range(min(HP, HP_LIM) if not SKIP_ATT else 0):
              wq = load_blk(wB, qk_d.ap(), 16, hp, KD * 128, tag="wq", bufs=2,
                            name=f"wq{hp}")
              wk = load_blk(wB, qk_d.ap(), 16, 8 + hp, KD * 128, tag="wk", bufs=2,
                            name=f"wk{hp}")
              rots = {}
              for which, wt in (("q", wq), ("k", wk)):
                  w3 = wt[:].rearrange("p (k m) -> p k m", k=KD)
                  rot = qkB.tile([128, S], bf16, name=f"{which}rot",
                                 tag=f"{which}rot", bufs=2)
                  pq2 = psB.tile([128, 2 * CH], f32, name="pq2", tag="W", bufs=2)
                  for kp in range(KP):
                      for c in range(NCH):
                          nc.tensor.matmul(pq2[:, c * CH:(c + 1) * CH],
                                           w3[:, 2 * kp:2 * kp + 2, :],
                                           xn3[:, 2 * kp:2 * kp + 2,
                                               c * CH:(c + 1) * CH],
                                           start=(kp == 0), stop=(kp == KP - 1),
                                           perf_mode=PM.DoubleRow)
                  sbw = qkB.tile([128, S], bf16, name="sbw", tag="sbw", bufs=2)
                  nc.vector.tensor_copy(sbw[:], pq2[:])
                  cosT = rope_t["cosq" if which == "q" else "cosk"]
                  sinT = rope_t["sinq" if which == "q" else "sink"]
                  psw2 = psB.tile([128, 2 * CH], f32, name="psw2", tag="W", bufs=2)
                  for c in range(NCH):
                      cs = slice(c * CH, (c + 1) * CH)
                      nc.tensor.matmul(psw2[:, c * CH:(c + 1) * CH], permt[:],
                                       sbw[:, cs], start=True, stop=True)
                      nc.vector.scalar_tensor_tensor(rot[:, cs], sbw[:, cs], 1.0,
                                                     cosT[:, cs],
                                                     op0=OP.mult, op1=OP.mult)
                      t2 = qkB.tile([128, CH], bf16, name="ropet2", tag="ropet2",
                                    bufs=2)
                      nc.vector.scalar_tensor_tensor(t2[:], psw2[:, c * CH:(c + 1) * CH],
                                                     1.0, sinT[:, cs],
                                                     op0=OP.mult, op1=OP.mult)
                      nc.vector.scalar_tensor_tensor(rot[:, cs], t2[:], 1.0,
                                                     rot[:, cs],
                                                     op0=OP.mult, op1=OP.add)
                  rots[which] = rot

              if STOP == "BQ":
                  for which in rots:
                      nc.sync.dma_start(outT_d.ap()[(0 if which == "q" else 128):(128 if which == "q" else 256), 0:512],
                                        rots[which][:, 0:1024].bitcast(f32)[:, 0:512])
                  break
              # both heads of the pair interleaved: their K=64 scores matmuls
              # sit on distinct PE row-groups (base partition 0 / 64) and pack.
              # psc covers both seq chunks of one key s-tile so each exp is one
              # [128,1024] ACT op; AV accumulates per (head, chunk) over s-tiles
              pavs = [psB.tile([128, CH], f32, name=f"pav{hh}{c}",
                               tag="av", bufs=4)
                      for hh in range(2) for c in range(NCH)]
              for skt in range(ST):
                  ex2 = [qkB.tile([128, 2 * CH], f8, name=f"ex{hh}",
                                  tag=f"ex{hh}", bufs=2) for hh in range(2)]
                  psc2 = [psB.tile([128, 2 * CH], f32, name="psc2",
                                   tag="W", bufs=2) for hh in range(2)]
                  for c in range(NCH):
                      for hh in range(2):
                          hs = slice(64 * hh, 64 * hh + 64)
                          nc.tensor.matmul(psc2[hh][:, c * CH:(c + 1) * CH],
                                           rots["k"][hs, skt * 128:(skt + 1) * 128],
                                           rots["q"][hs, c * CH:(c + 1) * CH],
                                           start=True, stop=True)
                  for hh in range(2):
                      nc.scalar.activation(ex2[hh][:], psc2[hh][:], AF.Exp)
                      h = 2 * hp + hh
                      lhsT = v2[skt // 2][:].rearrange("p (k x) -> p k x", k=2)[
                          :, skt % 2, h * VS:(h + 1) * VS]
                      for c in range(NCH):
                          nc.tensor.matmul(pavs[2 * hh + c][:], lhsT,
                                           ex2[hh][:, c * CH:(c + 1) * CH],
                                           start=(skt == 0), stop=(skt == ST - 1))
              for hh in range(2):
                  for c in range(NCH):
                      cs = slice(c * CH, (c + 1) * CH)
                      pav = pavs[2 * hh + c]
                      den = qkB.tile([64, CH], f32, name="den", tag="den", bufs=2)
                      nc.vector.reciprocal_approx_fast(den[:], pav[DH:2 * DH, :])
                      # head h = 2*hp+hh lands at k-tile hp, partitions
                      # [64*hh, 64*hh+64) of the attn feature layout; v carries
                      # 32x, denom 2x -> ratio is 16 * true attn
                      nc.vector.tensor_mul(at3[64 * hh:64 * hh + 64, hp, cs],
                                           pav[0:DH, :], den[:])
          if STOP in ("BQ", "B"):
              if STOP == "B":
                  nc.sync.dma_start(outT_d.ap()[0:128, :], at_f8[:, 0:4096].bitcast(f32))
              psB.release(); vB.release(); qkB.release(); wB.release(); xnP.release(); ropeP.release()
              atP.release(); x1pool.release(); cpool.release()
              nc.compile(); return nc
          psB.release()
          vB.release()
          qkB.release()
          wB.release()
          xnP.release()
          ropeP.release()

          # ---------------- phase C: out proj + residual + norm2 ---------
          wC = tc.alloc_tile_pool(name=f"wC{rep_i}", bufs=1)
          srcC = tc.alloc_tile_pool(name=f"srcC{rep_i}", bufs=1)
          psC = tc.alloc_tile_pool(name=f"psC{rep_i}", bufs=1, space="PSUM")
          for et in range(KD):
              wo = load_blk(wC, outw_d.ap(), KD, et, KD * 128, tag="wo", bufs=2,
                            name=f"wo{et}")
              wo3 = wo[:].rearrange("p (k m) -> p k m", k=KD)
              sc_t = srcC.tile([128, S], f32, name="srcCt", tag="srcCt", bufs=2)
              nc.sync.dma_start(sc_t[:], srcT_d.ap()[et * 128:(et + 1) * 128, :])
              po2 = psC.tile([128, 2 * CH], f32, name="po2", tag="wideC", bufs=2)
              for kp in range(KP):
                  for c in range(NCH):
                      nc.tensor.matmul(po2[:, c * CH:(c + 1) * CH],
                                       wo3[:, 2 * kp:2 * kp + 2, :],
                                       at3[:, 2 * kp:2 * kp + 2, c * CH:(c + 1) * CH],
                                       start=(kp == 0), stop=(kp == KP - 1),
                                       perf_mode=PM.DoubleRow)
              for c in range(NCH):
                  cs = slice(c * CH, (c + 1) * CH)
                  nc.vector.scalar_tensor_tensor(x1T[et][:, cs],
                                                 po2[:, c * CH:(c + 1) * CH],
                                                 g1c[:, et:et + 1], sc_t[:, cs],
                                                 op0=OP.mult, op1=OP.add)
          psC.release()
          srcC.release()
          wC.release()
          atP.release()

          # norm2 -> xn2_f8 (= x1 * rstd * 16)
          xn2P = tc.alloc_tile_pool(name=f"xn2P{rep_i}", bufs=1)
          xn2_f8 = xn2P.tile([128, KD * S], f8, name="xn2_f8", tag="xn2_f8")
          xn23 = xn2_f8[:].rearrange("p (k s) -> p k s", k=KD)
          psN2 = tc.alloc_tile_pool(name=f"psN2{rep_i}", bufs=1, space="PSUM")
          sqN2 = tc.alloc_tile_pool(name=f"sqN2{rep_i}", bufs=1)
          for c in range(NCH):
              cs = slice(c * CH, (c + 1) * CH)
              vrow2 = psN2.tile([1, CH], f32, name="vrow2", tag="vrow2", bufs=2)
              for kt in range(KD):
                  sq2 = sqN2.tile([128, CH], f32r, name="sq2", tag="sq2", bufs=3)
                  nc.vector.scalar_tensor_tensor(sq2[:], x1T[kt][:, cs], 1.0,
                                                 x1T[kt][:, cs],
                                                 op0=OP.mult, op1=OP.mult)
                  nc.tensor.matmul(vrow2[:], ones_col[:], sq2[:],
                                   start=(kt == 0), stop=(kt == KD - 1))
              srr2 = sqN2.tile([1, 2 * CH], f32r, name="srr2", tag="srr2", bufs=2)
              srow2 = srr2[0:1, 0:CH]
              rrow2 = srr2[0:1, CH:2 * CH]
              nc.scalar.activation(srow2, vrow2[:], AF.Sqrt,
                                   bias=eps1[:1, 0:1], scale=1.0 / D)
              with nc.allow_low_precision(reason="rstd fp32r rounding ok"):
                  nc.vector.reciprocal(rrow2, srow2)
              bc2 = psN2.tile([128, CH], f32, name="bc2", tag="bc2", bufs=2)
              nc.tensor.matmul(bc2[:], row16[:1, 0:128], rrow2, start=True, stop=True)
              bc2s = sqN2.tile([128, CH], f32, name="bc2s", tag="bc2s", bufs=2)
              nc.scalar.activation(bc2s[:], bc2[:], AF.Copy)
              for kt in range(KD):
                  nc.vector.scalar_tensor_tensor(xn23[:, kt, cs], x1T[kt][:, cs],
                                                 1.0, bc2s[:],
                                                 op0=OP.mult, op1=OP.mult)
          sqN2.release()
          psN2.release()
          if STOP == "C":
              for dt in range(KD):
                  nc.sync.dma_start(outT_d.ap()[dt * 128:(dt + 1) * 128, :], x1T[dt][:])
              xn2P.release(); x1pool.release(); cpool.release()
              nc.compile(); return nc

          # ---------------- phase D: FFN (shared + experts, fused) -------
          # hbuf holds all 96 h-tiles (scaled by 8 * routing weight); the down
          # projection accumulates rounds of 48 h-tiles straight into x1T.
          wD = tc.alloc_tile_pool(name=f"wD{rep_i}", bufs=1)
          hD = tc.alloc_tile_pool(name=f"hD{rep_i}", bufs=1)
          psD = tc.alloc_tile_pool(name=f"psD{rep_i}", bufs=1, space="PSUM")
          hbufs = [hD.tile([128, ROUND * S], f8, name=f"hbuf{r}", tag=f"hbuf{r}")
                   for r in range(2)]
          hb3s = [h[:].rearrange("p (i s) -> p i s", i=ROUND) for h in hbufs]

          def hb_slot(slot):
              return hb3s[slot // ROUND], slot % ROUND

          def gate_up(wg_ap, wg_nb, wg_b, wu_ap, wu_nb, wu_b, slot):
              """hbuf[slot] = 8 * rw * silu(xn2 @ wg) * (xn2 @ wu)"""
              wg = load_blk(wD, wg_ap, wg_nb, wg_b, KD * 128, tag="wg", bufs=3,
                            name="wg")
              wu = load_blk(wD, wu_ap, wu_nb, wu_b, KD * 128, tag="wu", bufs=3,
                            name="wu")
              wg3 = wg[:].rearrange("p (k m) -> p k m", k=KD)
              wu3 = wu[:].rearrange("p (k m) -> p k m", k=KD)
              pg2 = psD.tile([128, 2 * CH], f32, name="pg2", tag="ps", bufs=4)
              pu2 = psD.tile([128, 2 * CH], f32, name="pu2", tag="ps", bufs=4)
              for kp in range(KP):
                  for c in range(NCH):
                      nc.tensor.matmul(pg2[:, c * CH:(c + 1) * CH],
                                       wg3[:, 2 * kp:2 * kp + 2, :],
                                       xn23[:, 2 * kp:2 * kp + 2, c * CH:(c + 1) * CH],
                                       start=(kp == 0), stop=(kp == KP - 1),
                                       perf_mode=PM.DoubleRow)
              for kp in range(KP):
                  for c in range(NCH):
                      nc.tensor.matmul(pu2[:, c * CH:(c + 1) * CH],
                                       wu3[:, 2 * kp:2 * kp + 2, :],
                                       xn23[:, 2 * kp:2 * kp + 2, c * CH:(c + 1) * CH],
                                       start=(kp == 0), stop=(kp == KP - 1),
                                       perf_mode=PM.DoubleRow)
              g = slot // GRP
              hb, si = hb_slot(slot)
              sg = hD.tile([128, 2 * CH], f32r, name="sg", tag="sg", bufs=2)
              nc.scalar.activation(sg[:], pg2[:], AF.Silu,
                                   scale=2.0 ** (-(SX + SW)))
              nc.vector.scalar_tensor_tensor(hb[:, si, :], pu2[:],
                                             dsc[:, g:g + 1],
                                             sg[:], op0=OP.mult, op1=OP.mult)

          def down(r):
              """x1T += g2 * (hbuf[r*48:(r+1)*48] @ wd)"""
              for dt in range(KD):
                  wd = load_blk(wD, wd_d.ap(), KD * 2, dt * 2 + r, ROUND * 128,
                                tag="wd", bufs=2, name="wd")
                  wd3 = wd[:].rearrange("p (i m) -> p i m", i=ROUND)
                  pd2 = psD.tile([128, 2 * CH], f32, name="pd2", tag="ps", bufs=4)
                  for ip in range(ROUND // 2):
                      for c in range(NCH):
                          nc.tensor.matmul(pd2[:, c * CH:(c + 1) * CH],
                                           wd3[:, 2 * ip:2 * ip + 2, :],
                                           hb3s[r][:, 2 * ip:2 * ip + 2,
                                                   c * CH:(c + 1) * CH],
                                           start=(ip == 0), stop=(ip == ROUND // 2 - 1),
                                           perf_mode=PM.DoubleRow)
                  nc.vector.scalar_tensor_tensor(x1T[dt][:, :], pd2[:],
                                                 g2c[:, dt:dt + 1],
                                                 x1T[dt][:, :],
                                                 op0=OP.mult, op1=OP.add)
                  if r == 1:
                      nc.sync.dma_start(outT_d.ap()[dt * 128:(dt + 1) * 128, :],
                                        x1T[dt][:])

          GRP = 8
          ht_args = []
          for g in range(SHT // GRP):
              for i in range(GRP):
                  ht_args.append((shwg_d.ap(), SHT, g * GRP + i,
                                  shwu_d.ap(), SHT, g * GRP + i))
          for e in range(NE):
              for i in range(RHT):
                  ht_args.append((rwg_d.ap(), NE * RHT, e * RHT + i,
                                  rwu_d.ap(), NE * RHT, e * RHT + i))
          if SKIP_FFN:
              for dt in range(KD):
                  nc.sync.dma_start(outT_d.ap()[dt * 128:(dt + 1) * 128, :],
                                    x1T[dt][:])
          else:
              for slot, args in enumerate(ht_args):
                  gate_up(*args, slot)
                  if slot == ROUND - 1:
                      down(0)
              down(1)
          psD.release()
          hD.release()
          wD.release()
          xn2P.release()
          x1pool.release()
          cpool.release()

    nc.compile()
    return nc


# ---------------------------------------------------------------- entry point

_CACHE = {}

_IN_ORDER = ["src", "centroids", "routing_weights", "qkv_w", "qkv_b", "out_w",
             "out_b", "norm1_w", "norm2_w", "gamma_1", "gamma_2",
             "sh_wg", "sh_bg", "sh_wu", "sh_bu", "sh_wd", "sh_bd",
             "r_wg", "r_bg", "r_wu", "r_bu", "r_wd", "r_bd"]


def _prep(cfg, inputs):
    args = [np.asarray(inputs[k]) for k in _IN_ORDER]
    return host_prep(cfg, *args)


def kernel(**inputs):
    cfg = FULL
    in_maps = _prep(cfg, inputs)
    if "nc" not in _CACHE:
        _CACHE["nc"] = build_nc(cfg)
    nc = _CACHE["nc"]
    res = bass_utils.run_bass_kernel_spmd(nc, in_maps, core_ids=list(range(cfg["NCORES"])))
    B, S, D = np.asarray(inputs["src"]).shape
    out = np.empty((B, S, D), np.float32)
    for b in range(B):
        out[b] = res.results[b]["outT"].T
    return out


# revision 28
# speedup vs baseline: 1.0785x; 1.0785x over previous
"""MoE transformer layer (soft-routed) on 8 TRN2 NeuronCores.

Strategy: pure batch-data-parallel (B == n_cores == 8); each core runs the
full layer for one batch element, no collectives. All heavy matmuls run in
fp8e4 DoubleRow mode (2 fp8 weights per PE cell -> 0.5 cycles/row, 2x the
fp32r/bf16 rate). Accumulation stays fp32 in PSUM. The layer-scale structure
(gamma = 1e-5 on both residual branches) leaves enormous numerical headroom,
so fp8/bf16 quantization of weights/activations is far inside tolerance.

Engine balance (per the instruction cost model):
  - PE: all projections fp8 DoubleRow, 2 moving matmuls per LDWEIGHTS pair.
  - ACT: softmax exp of 3 in 4 heads, silu, PSUM->SBUF wide copies.
  - DVE: epilogue scalar_tensor_tensor ops (2x all-SBUF mode where possible,
    bf16 rope chain for the 2x 16-bit mode), exp of 1 in 4 heads via pow.
  - The whole routed+shared FFN down-projection accumulates in PSUM across
    all 96 h-tiles (routing weights folded into the gate epilogue), so the
    down epilogue is one scalar_tensor_tensor per (dt, seq-half, round).

Scale bookkeeping (power-of-2, folded into existing epilogues at zero cost):
  weights * 2^11; xn, xn2, attn * 16 (via a 16.0-row in the rstd/softmax
  broadcast matmuls); v * 32 (softmax-denom column is 32 too -> ratio exact);
  hidden * 8 * routing_weight (via the per-group dscale column). Dequants live
  in the cos/sin tables (q also carries 1/sqrt(dh)), gamma columns, and the
  silu activation scale.
"""
import sys, os

for _p in ("/opt/trn_rl_repo", "/root/.axon_site/_ro/trn_rl_repo"):
    if os.path.isdir(_p) and _p not in sys.path:
        sys.path.insert(0, _p)

import numpy as np
import ml_dtypes
import concourse.bacc as bacc
import concourse.mybir as mybir
from concourse import bass_utils
from concourse.tile import TileContext

f32 = mybir.dt.float32
f32r = mybir.dt.float32r
bf16 = mybir.dt.bfloat16
f8 = mybir.dt.float8e4
AF = mybir.ActivationFunctionType
OP = mybir.AluOpType
PM = mybir.MatmulPerfMode
NPF8 = ml_dtypes.float8_e4m3   # TRN FP8_EXP4: max +-240, has inf
NPBF = ml_dtypes.bfloat16

FULL = dict(D=1024, S=1024, NH=16, SH=4096, RH=1024, NE=8, NCORES=8)
EPS = 1e-5
ROPE_SCALES = (1.0, 1.0, 1.0, 0.2)
ROPE_BASE = 10000.0

SW = 11        # weight scale exponent (all weight tensors)
SX = 4         # xn / xn2 / attn activation scale exponent (16)
SV = 5         # v scale exponent (32)
SHID = 3       # ffn hidden scale exponent (8)


def _to_f8(a):
    return np.clip(a, -240.0, 240.0).astype(NPF8)


def _pack_w(WT, mw, scale=float(2 ** SW)):
    """WT [in_f, out_f] fp32 -> fp8 packed [128, nb*kd*mw] with per-partition
    layout [b, k, m]: block b covers output cols [b*mw,(b+1)*mw), k indexes
    the 128-row contraction tiles. A block DMA is 128 lines of kd*mw bytes."""
    in_f, out_f = WT.shape
    kd, nb = in_f // 128, out_f // mw
    a = (np.asarray(WT, np.float64) * scale).reshape(kd, 128, nb, mw)
    a = a.transpose(1, 2, 0, 3).reshape(128, nb * kd * mw)
    return _to_f8(a)


# ---------------------------------------------------------------- host prep

def host_prep(cfg, src, centroids, routing_weights, qkv_w, qkv_b, out_w, out_b,
              norm1_w, norm2_w, gamma_1, gamma_2,
              sh_wg, sh_bg, sh_wu, sh_bu, sh_wd, sh_bd,
              r_wg, r_bg, r_wu, r_bu, r_wd, r_bd):
    """Returns list of per-core input dicts (one batch element per core)."""
    D, S, NH, SH, RH, NE = cfg["D"], cfg["S"], cfg["NH"], cfg["SH"], cfg["RH"], cfg["NE"]
    DH = D // NH
    KD = D // 128
    B = src.shape[0]
    c = lambda a: np.ascontiguousarray(a, dtype=np.float32)
    cb = lambda a: np.ascontiguousarray(np.asarray(a, np.float32).astype(NPBF))
    scale = 1.0 / np.sqrt(DH)

    assert not np.any(qkv_b) and not np.any(out_b), "nonzero qkv/out bias unsupported"
    assert not np.any(sh_bd) and not np.any(r_bd), "nonzero down-proj bias unsupported"
    assert not (np.any(sh_bg) or np.any(sh_bu) or np.any(r_bg) or np.any(r_bu)), \
        "nonzero gate/up bias unsupported"

    qkvT = qkv_w.T * norm1_w[:, None]          # [D, 3D], no score-scale fold
    shwgT = sh_wg.T * norm2_w[:, None]
    shwuT = sh_wu.T * norm2_w[:, None]

    # down: all 96 h-tiles (shared 0..31, expert e at 32+8e) in one matrix,
    # packed per (dt, round r of 48 h-tiles): [128, 48i, 128j]
    big_wd = np.concatenate([np.asarray(sh_wd).T]
                            + [np.asarray(r_wd[e]).T for e in range(NE)], axis=0)
    wd_all = _to_f8((big_wd * 2.0 ** SW).reshape(2, 48, 128, 8, 128)
                    .transpose(2, 3, 0, 1, 4).reshape(128, -1))

    shared = {
        "qk_w": _pack_w(qkvT[:, : 2 * D], 128),          # blocks: q hp 0..7, k 8..15
        "v_w": _pack_w(qkvT[:, 2 * D:], 512),            # 2 blocks of 8 heads
        "out_w8": _pack_w(out_w.T, 128),                 # 8 blocks (et)
        "g1_col": c((np.asarray(gamma_1) * 2.0 ** (-(SX + SW))).reshape(KD, 128).T),
        "g2_col": c((np.asarray(gamma_2) * 2.0 ** (-(SHID + SW))).reshape(KD, 128).T),
        "sh_wg8": _pack_w(shwgT, 128),                   # 32 blocks (ht)
        "sh_wu8": _pack_w(shwuT, 128),
        "wd_all": wd_all,
        "r_wg8": np.ascontiguousarray(
            np.stack([_pack_w(r_wg[e].T * norm2_w[:, None], 128) for e in range(NE)])
            .reshape(NE, 128, -1).transpose(1, 0, 2).reshape(128, -1)),
        "r_wu8": np.ascontiguousarray(
            np.stack([_pack_w(r_wu[e].T * norm2_w[:, None], 128) for e in range(NE)])
            .reshape(NE, 128, -1).transpose(1, 0, 2).reshape(128, -1)),
    }
    perm = np.zeros((128, 128), np.float32)
    perm[np.arange(128) ^ 8, np.arange(128)] = 1.0
    shared["perm"] = cb(perm)

    # rope tables: row p handles head-local feature dh = p % DH
    p = np.arange(128)
    dh = p % DH
    d_axis = DH // 4
    half = d_axis // 2
    a_idx = dh // d_axis
    j = dh % d_axis
    f = j % half
    sign = np.where(j < half, -1.0, 1.0).astype(np.float32)
    inv_freq = (1.0 / (ROPE_BASE ** (np.arange(half) / half))).astype(np.float32)

    dq = scale * 2.0 ** (-(SX + SW))   # q dequant incl. 1/sqrt(dh)
    dk = 2.0 ** (-(SX + SW))
    in_maps = []
    for b in range(B):
        m = dict(shared)
        m["srcT"] = c(np.asarray(src[b]).T)
        pos = np.asarray(centroids[b])[:, a_idx] * np.asarray(ROPE_SCALES, np.float32)[a_idx]
        ang = (pos * inv_freq[f][None, :]).T.astype(np.float32)        # [128, S]
        cosv, sinv = np.cos(ang), sign[:, None] * np.sin(ang)
        m["cosqT"], m["sinqT"] = cb(cosv * dq), cb(sinv * dq)
        m["coskT"], m["sinkT"] = cb(cosv * dk), cb(sinv * dk)
        rw = np.asarray(routing_weights[b], np.float32)
        ds = np.empty((12,), np.float32)
        ds[:4] = 2.0 ** (SHID - SX - SW)
        ds[4:] = rw * 2.0 ** (SHID - SX - SW)
        m["dscale"] = c(np.broadcast_to(ds, (128, 12)))
        in_maps.append(m)
    return in_maps


# ---------------------------------------------------------------- device build

def build_nc(cfg, reps=1):
    STOP = os.environ.get("STOP_AFTER", "")
    SKIP_ATT = bool(os.environ.get("SKIP_ATT"))
    HP_LIM = int(os.environ.get("HP_LIM", "8"))
    SKIP_FFN = bool(os.environ.get("SKIP_FFN"))
    D, S, NH, SH, RH, NE = cfg["D"], cfg["S"], cfg["NH"], cfg["SH"], cfg["RH"], cfg["NE"]
    DH = D // NH
    assert DH == 64 and D % 128 == 0 and S % 128 == 0 and SH % 128 == 0 and RH % 128 == 0
    KD, ST, SHT, RHT = D // 128, S // 128, SH // 128, RH // 128
    CH = min(512, S)
    NCH = S // CH
    HP = NH // 2          # head pairs
    VS = 2 * DH           # v stride per head (64 data + 64 denom cols of 2.0)
    KP = KD // 2          # contraction k-tile pairs
    NHT = SHT + NE * RHT  # 96 total h-tiles
    ROUND = NHT // 2      # h-tiles per down round

    nc = bacc.Bacc("TRN2", target_bir_lowering=False, debug=False)

    def din(name, shape, dt=f32):
        return nc.dram_tensor(name, list(shape), dt, kind="ExternalInput")

    srcT_d = din("srcT", (D, S))
    cosq_d = din("cosqT", (128, S), bf16)
    sinq_d = din("sinqT", (128, S), bf16)
    cosk_d = din("coskT", (128, S), bf16)
    sink_d = din("sinkT", (128, S), bf16)
    perm_d = din("perm", (128, 128), bf16)
    qk_d = din("qk_w", (128, 16 * KD * 128), f8)
    v_d = din("v_w", (128, 2 * KD * 512), f8)
    outw_d = din("out_w8", (128, KD * KD * 128), f8)
    g1_d = din("g1_col", (128, KD))
    g2_d = din("g2_col", (128, KD))
    shwg_d = din("sh_wg8", (128, SHT * KD * 128), f8)
    shwu_d = din("sh_wu8", (128, SHT * KD * 128), f8)
    wd_d = din("wd_all", (128, KD * 2 * ROUND * 128), f8)
    rwg_d = din("r_wg8", (128, NE * RHT * KD * 128), f8)
    rwu_d = din("r_wu8", (128, NE * RHT * KD * 128), f8)
    dsc_d = din("dscale", (128, 12))
    outT_d = nc.dram_tensor("outT", [D, S], f32, kind="ExternalOutput")

    def load_blk(pool, dram_ap, nb, b, width, tag, bufs, name):
        """Load packed weight block b: [128, width] contiguous, fp8."""
        t = pool.tile([128, width], f8, name=name, tag=tag, bufs=bufs)
        src = dram_ap.rearrange("p (b x) -> p b x", b=nb)[:, b, :]
        nc.sync.dma_start(t[:], src)
        return t

    with TileContext(nc) as tc:
      for rep_i in range(reps):
          cpool = tc.alloc_tile_pool(name=f"const{rep_i}", bufs=1)

          onesf = cpool.tile([128, 32], f32, name="onesf", tag="onesf")
          nc.vector.memset(onesf[:], 1.0)
          c2f8 = cpool.tile([128, 64], f8, name="c2f8", tag="c2f8")
          nc.vector.memset(c2f8[:], 2.0)
          row16f = cpool.tile([1, 128], f32, name="row16f", tag="row16f")
          nc.vector.memset(row16f[:], 16.0)
          row16 = cpool.tile([1, 128], f32r, name="row16", tag="row16")
          nc.vector.tensor_copy(row16[:], row16f[:])
          ones_col = cpool.tile([128, 1], f32r, name="ones_col", tag="ones_col")
          nc.vector.tensor_copy(ones_col[:], onesf[:, 0:1])
          eps1 = cpool.tile([1, 1], f32, name="eps1", tag="eps1")
          nc.vector.memset(eps1[:], EPS)
          e_t = cpool.tile([128, 2 * CH], f32, name="e_t", tag="e_t")
          nc.vector.memset(e_t[:], float(np.e))
          permt = cpool.tile([128, 128], bf16, name="permt", tag="permt")
          nc.sync.dma_start(permt[:], perm_d.ap())
          g1c = cpool.tile([128, KD], f32, name="g1c", tag="g1c")
          nc.sync.dma_start(g1c[:], g1_d.ap())
          g2c = cpool.tile([128, KD], f32, name="g2c", tag="g2c")
          nc.sync.dma_start(g2c[:], g2_d.ap())
          dsc = cpool.tile([128, 12], f32, name="dsc", tag="dsc")
          nc.sync.dma_start(dsc[:], dsc_d.ap())

          x1pool = tc.alloc_tile_pool(name=f"x1p{rep_i}", bufs=1)
          x1T = [x1pool.tile([128, S], f32, name=f"x1T{kt}", tag=f"x1T{kt}")
                 for kt in range(KD)]

          # attn output in fp8 [k, s] layout (lives until phase C)
          atP = tc.alloc_tile_pool(name=f"atP{rep_i}", bufs=1)
          at_f8 = atP.tile([128, KD * S], f8, name="at_f8", tag="at_f8")
          at3 = at_f8[:].rearrange("p (k s) -> p k s", k=KD)

          # rope tables (bf16), released after attention
          ropeP = tc.alloc_tile_pool(name=f"ropeP{rep_i}", bufs=1)
          rope_t = {}
          for nm, d in (("cosq", cosq_d), ("sinq", sinq_d),
                        ("cosk", cosk_d), ("sink", sink_d)):
              t = ropeP.tile([128, S], bf16, name=nm, tag=nm)
              nc.sync.dma_start(t[:], d.ap())
              rope_t[nm] = t

          # ---------------- phase A: rms norm 1 -> xn_f8 (= src * rstd * 16)
          xnP = tc.alloc_tile_pool(name=f"xnP{rep_i}", bufs=1)
          xn_f8 = xnP.tile([128, KD * S], f8, name="xn_f8", tag="xn_f8")
          xn3 = xn_f8[:].rearrange("p (k s) -> p k s", k=KD)

          srcA = tc.alloc_tile_pool(name=f"srcA{rep_i}", bufs=1)
          psA = tc.alloc_tile_pool(name=f"psA{rep_i}", bufs=1, space="PSUM")
          sqA = tc.alloc_tile_pool(name=f"sqA{rep_i}", bufs=1)
          srcT = []
          for kt in range(KD):
              t = srcA.tile([128, S], f32, name=f"srcT{kt}", tag=f"srcT{kt}")
              nc.sync.dma_start(t[:], srcT_d.ap()[kt * 128:(kt + 1) * 128, :])
              srcT.append(t)
          for c in range(NCH):
              cs = slice(c * CH, (c + 1) * CH)
              vrow_ps = psA.tile([1, CH], f32, name="vrow_ps", tag="vrow", bufs=2)
              for kt in range(KD):
                  sq = sqA.tile([128, CH], f32r, name="sq", tag="sq", bufs=3)
                  nc.vector.scalar_tensor_tensor(sq[:], srcT[kt][:, cs], 1.0,
                                                 srcT[kt][:, cs],
                                                 op0=OP.mult, op1=OP.mult)
                  nc.tensor.matmul(vrow_ps[:], ones_col[:], sq[:],
                                   start=(kt == 0), stop=(kt == KD - 1))
              srr = sqA.tile([1, 2 * CH], f32r, name="srr", tag="srr", bufs=2)
              srow = srr[0:1, 0:CH]
              rrow = srr[0:1, CH:2 * CH]
              nc.scalar.activation(srow, vrow_ps[:], AF.Sqrt,
                                   bias=eps1[:1, 0:1], scale=1.0 / D)
              with nc.allow_low_precision(reason="rstd fp32r rounding ok"):
                  nc.vector.reciprocal(rrow, srow)
              bc = psA.tile([128, CH], f32, name="bcA", tag="bcA", bufs=2)
              nc.tensor.matmul(bc[:], row16[:1, 0:128], rrow, start=True, stop=True)
              bcs = sqA.tile([128, CH], f32, name="bcsA", tag="bcsA", bufs=2)
              nc.scalar.activation(bcs[:], bc[:], AF.Copy)
              for kt in range(KD):
                  nc.vector.scalar_tensor_tensor(xn3[:, kt, cs], srcT[kt][:, cs],
                                                 1.0, bcs[:],
                                                 op0=OP.mult, op1=OP.mult)
          sqA.release()
          psA.release()
          srcA.release()
          if STOP == "A":
              nc.sync.dma_start(outT_d.ap()[0:128, :], xn_f8[:, 0:4096].bitcast(f32))
              xnP.release(); ropeP.release(); atP.release(); x1pool.release(); cpool.release()
              nc.compile(); return nc

          # ---------------- phase B: attention --------------------------
          if SKIP_ATT or HP_LIM < 8:
              nc.vector.memset(at_f8[:], 0.125)
          wB = tc.alloc_tile_pool(name=f"wB{rep_i}", bufs=1)
          qkB = tc.alloc_tile_pool(name=f"qkB{rep_i}", bufs=1)
          vB = tc.alloc_tile_pool(name=f"vB{rep_i}", bufs=1)
          psB = tc.alloc_tile_pool(name=f"psB{rep_i}", bufs=1, space="PSUM")

          # ---- v in [s-pair, 2, heads*VS] fp8 layout, denom col = 32 ----
          v2 = [vB.tile([128, 2 * NH * VS], f8, name=f"v{pr}", tag=f"v{pr}")
                for pr in range(ST // 2)]
          for pr in range(ST // 2 if not SKIP_ATT else 0):
              oc = v2[pr][:].rearrange("p (k h c) -> p k h c", k=2, c=VS)[:, :, :, DH:2 * DH]
              nc.vector.tensor_copy(
                  oc, c2f8[:, None, None, :].to_broadcast((128, 2, NH, DH)))
          wv = [load_blk(wB, v_d.ap(), 2, vb, KD * 512, tag=f"wv{vb}", bufs=1,
                         name=f"wv{vb}") for vb in range(2)] if not SKIP_ATT else []
          wv3 = [w[:].rearrange("p (k m) -> p k m", k=KD) for w in wv]
          for st in range(ST if not SKIP_ATT else 0):
              pv2 = psB.tile([128, 2 * CH], f32, name="pv2", tag="W", bufs=2)
              for kp in range(KP):
                  lhsT = xn3[:, 2 * kp:2 * kp + 2, st * 128:(st + 1) * 128]
                  for vb in range(2):
                      nc.tensor.matmul(pv2[:, vb * CH:(vb + 1) * CH], lhsT,
                                       wv3[vb][:, 2 * kp:2 * kp + 2, :],
                                       start=(kp == 0), stop=(kp == KP - 1),
                                       perf_mode=PM.DoubleRow)
              dst = v2[st // 2][:].rearrange("p (k h c) -> p k h c", k=2, c=VS)[
                  :, st % 2, :, 0:DH]
              nc.scalar.activation(dst, pv2[:].rearrange("p (h c) -> p h c", c=DH),
                                   AF.Copy, scale=2.0 ** (SV - SX - SW))

          if STOP == "BV":
              nc.sync.dma_start(outT_d.ap()[0:128, :], v2[0][:, 0:2048].bitcast(f32)[:, 0:1024])
              psB.release(); vB.release(); qkB.release(); wB.release(); xnP.release(); ropeP.release()
              atP.release(); x1pool.release(); cpool.release()
              nc.compile(); return nc

          # ---- per head pair: qk proj, rope, scores, av ------------
          all_rots = []
          for hp in range(min(HP, HP_LIM) if not SKIP_ATT else 0):
              wq = load_blk(wB, qk_d.ap(), 16, hp, KD * 128, tag="wq", bufs=2,
                            name=f"wq{hp}")
              wk = load_blk(wB, qk_d.ap(), 16, 8 + hp, KD * 128, tag="wk", bufs=2,
                            name=f"wk{hp}")
              rots = {}
              for which, wt in (("q", wq), ("k", wk)):
                  w3 = wt[:].rearrange("p (k m) -> p k m", k=KD)
                  rot = qkB.tile([128, S], bf16, name=f"{which}rot{hp}",
                                 tag=f"{which}rot{hp}", bufs=1)
                  pq2 = psB.tile([128, 2 * CH], f32, name="pq2", tag="W", bufs=2)
                  for kp in range(KP):
                      for c in range(NCH):
                          nc.tensor.matmul(pq2[:, c * CH:(c + 1) * CH],
                                           w3[:, 2 * kp:2 * kp + 2, :],
                                           xn3[:, 2 * kp:2 * kp + 2,
                                               c * CH:(c + 1) * CH],
                                           start=(kp == 0), stop=(kp == KP - 1),
                                           perf_mode=PM.DoubleRow)
                  sbw = qkB.tile([128, S], bf16, name="sbw", tag="sbw", bufs=2)
                  nc.vector.tensor_copy(sbw[:], pq2[:])
                  cosT = rope_t["cosq" if which == "q" else "cosk"]
                  sinT = rope_t["sinq" if which == "q" else "sink"]
                  psw2 = psB.tile([128, 2 * CH], f32, name="psw2", tag="W", bufs=2)
                  for c in range(NCH):
                      cs = slice(c * CH, (c + 1) * CH)
                      nc.tensor.matmul(psw2[:, c * CH:(c + 1) * CH], permt[:],
                                       sbw[:, cs], start=True, stop=True)
                      nc.vector.scalar_tensor_tensor(rot[:, cs], sbw[:, cs], 1.0,
                                                     cosT[:, cs],
                                                     op0=OP.mult, op1=OP.mult)
                      t2 = qkB.tile([128, CH], bf16, name="ropet2", tag="ropet2",
                                    bufs=2)
                      nc.vector.scalar_tensor_tensor(t2[:], psw2[:, c * CH:(c + 1) * CH],
                                                     1.0, sinT[:, cs],
                                                     op0=OP.mult, op1=OP.mult)
                      nc.vector.scalar_tensor_tensor(rot[:, cs], t2[:], 1.0,
                                                     rot[:, cs],
                                                     op0=OP.mult, op1=OP.add)
                  rots[which] = rot
              all_rots.append(rots)

          for hp in range(len(all_rots)):
              rots = all_rots[hp]
              if STOP == "BQ":
                  for which in rots:
                      nc.sync.dma_start(outT_d.ap()[(0 if which == "q" else 128):(128 if which == "q" else 256), 0:512],
                                        rots[which][:, 0:1024].bitcast(f32)[:, 0:512])
                  break
              # both heads of the pair interleaved: their K=64 scores matmuls
              # sit on distinct PE row-groups (base partition 0 / 64) and pack.
              # psc covers both seq chunks of one key s-tile so each exp is one
              # [128,1024] ACT op; AV accumulates per (head, chunk) over s-tiles
              pavs = [psB.tile([128, CH], f32, name=f"pav{hh}{c}",
                               tag="av", bufs=4)
                      for hh in range(2) for c in range(NCH)]
              ex4 = [None, None]
              for skt in range(ST):
                  if skt % 2 == 0:
                      ex4 = [qkB.tile([128, 4 * CH], f8, name=f"ex{hh}",
                                      tag=f"ex{hh}", bufs=2) for hh in range(2)]
                  psc2 = [psB.tile([128, 2 * CH], f32, name="psc2",
                                   tag="W", bufs=2) for hh in range(2)]
                  for c in range(NCH):
                      for hh in range(2):
                          hs = slice(64 * hh, 64 * hh + 64)
                          nc.tensor.matmul(psc2[hh][:, c * CH:(c + 1) * CH],
                                           rots["k"][hs, skt * 128:(skt + 1) * 128],
                                           rots["q"][hs, c * CH:(c + 1) * CH],
                                           start=True, stop=True)
                  for hh in range(2):
                      sub = skt % 2
                      nc.scalar.activation(
                          ex4[hh][:, sub * 2 * CH:(sub + 1) * 2 * CH],
                          psc2[hh][:], AF.Exp)
                      if sub == 1:
                          h = 2 * hp + hh
                          lhsT = v2[skt // 2][:].rearrange("p (k x) -> p k x", k=2)[
                              :, :, h * VS:(h + 1) * VS]
                          ex3 = ex4[hh][:].rearrange("p (s k x) -> p s k x", s=2, k=2)
                          for c in range(NCH):
                              nc.tensor.matmul(pavs[2 * hh + c][:], lhsT,
                                               ex3[:, :, c, :].rearrange("p s x -> p s x"),
                                               start=(skt == 1), stop=(skt == ST - 1),
                                               perf_mode=PM.DoubleRow)
              for hh in range(2):
                  for c in range(NCH):
                      cs = slice(c * CH, (c + 1) * CH)
                      pav = pavs[2 * hh + c]
                      den = qkB.tile([64, CH], f32, name="den", tag="den", bufs=2)
                      nc.vector.reciprocal_approx_fast(den[:], pav[DH:2 * DH, :])
                      # head h = 2*hp+hh lands at k-tile hp, partitions
                      # [64*hh, 64*hh+64) of the attn feature layout; v carries
                      # 32x, denom 2x -> ratio is 16 * true attn
                      nc.vector.tensor_mul(at3[64 * hh:64 * hh + 64, hp, cs],
                                           pav[0:DH, :], den[:])
          if STOP in ("BQ", "B"):
              if STOP == "B":
                  nc.sync.dma_start(outT_d.ap()[0:128, :], at_f8[:, 0:4096].bitcast(f32))
              psB.release(); vB.release(); qkB.release(); wB.release(); xnP.release(); ropeP.release()
              atP.release(); x1pool.release(); cpool.release()
              nc.compile(); return nc
          psB.release()
          vB.release()
          qkB.release()
          wB.release()
          xnP.release()
          ropeP.release()

          # ---------------- phase C: out proj + residual + norm2 ---------
          wC = tc.alloc_tile_pool(name=f"wC{rep_i}", bufs=1)
          srcC = tc.alloc_tile_pool(name=f"srcC{rep_i}", bufs=1)
          psC = tc.alloc_tile_pool(name=f"psC{rep_i}", bufs=1, space="PSUM")
          for et in range(KD):
              wo = load_blk(wC, outw_d.ap(), KD, et, KD * 128, tag="wo", bufs=2,
                            name=f"wo{et}")
              wo3 = wo[:].rearrange("p (k m) -> p k m", k=KD)
              sc_t = srcC.tile([128, S], f32, name="srcCt", tag="srcCt", bufs=2)
              nc.sync.dma_start(sc_t[:], srcT_d.ap()[et * 128:(et + 1) * 128, :])
              po2 = psC.tile([128, 2 * CH], f32, name="po2", tag="wideC", bufs=2)
              for kp in range(KP):
                  for c in range(NCH):
                      nc.tensor.matmul(po2[:, c * CH:(c + 1) * CH],
                                       wo3[:, 2 * kp:2 * kp + 2, :],
                                       at3[:, 2 * kp:2 * kp + 2, c * CH:(c + 1) * CH],
                                       start=(kp == 0), stop=(kp == KP - 1),
                                       perf_mode=PM.DoubleRow)
              for c in range(NCH):
                  cs = slice(c * CH, (c + 1) * CH)
                  nc.vector.scalar_tensor_tensor(x1T[et][:, cs],
                                                 po2[:, c * CH:(c + 1) * CH],
                                                 g1c[:, et:et + 1], sc_t[:, cs],
                                                 op0=OP.mult, op1=OP.add)
          psC.release()
          srcC.release()
          wC.release()
          atP.release()

          # norm2 -> xn2_f8 (= x1 * rstd * 16)
          xn2P = tc.alloc_tile_pool(name=f"xn2P{rep_i}", bufs=1)
          xn2_f8 = xn2P.tile([128, KD * S], f8, name="xn2_f8", tag="xn2_f8")
          xn23 = xn2_f8[:].rearrange("p (k s) -> p k s", k=KD)
          psN2 = tc.alloc_tile_pool(name=f"psN2{rep_i}", bufs=1, space="PSUM")
          sqN2 = tc.alloc_tile_pool(name=f"sqN2{rep_i}", bufs=1)
          for c in range(NCH):
              cs = slice(c * CH, (c + 1) * CH)
              vrow2 = psN2.tile([1, CH], f32, name="vrow2", tag="vrow2", bufs=2)
              for kt in range(KD):
                  sq2 = sqN2.tile([128, CH], f32r, name="sq2", tag="sq2", bufs=3)
                  nc.vector.scalar_tensor_tensor(sq2[:], x1T[kt][:, cs], 1.0,
                                                 x1T[kt][:, cs],
                                                 op0=OP.mult, op1=OP.mult)
                  nc.tensor.matmul(vrow2[:], ones_col[:], sq2[:],
                                   start=(kt == 0), stop=(kt == KD - 1))
              srr2 = sqN2.tile([1, 2 * CH], f32r, name="srr2", tag="srr2", bufs=2)
              srow2 = srr2[0:1, 0:CH]
              rrow2 = srr2[0:1, CH:2 * CH]
              nc.scalar.activation(srow2, vrow2[:], AF.Sqrt,
                                   bias=eps1[:1, 0:1], scale=1.0 / D)
              with nc.allow_low_precision(reason="rstd fp32r rounding ok"):
                  nc.vector.reciprocal(rrow2, srow2)
              bc2 = psN2.tile([128, CH], f32, name="bc2", tag="bc2", bufs=2)
              nc.tensor.matmul(bc2[:], row16[:1, 0:128], rrow2, start=True, stop=True)
              bc2s = sqN2.tile([128, CH], f32, name="bc2s", tag="bc2s", bufs=2)
              nc.scalar.activation(bc2s[:], bc2[:], AF.Copy)
              for kt in range(KD):
                  nc.vector.scalar_tensor_tensor(xn23[:, kt, cs], x1T[kt][:, cs],
                                                 1.0, bc2s[:],
                                                 op0=OP.mult, op1=OP.mult)
          sqN2.release()
          psN2.release()
          if STOP == "C":
              for dt in range(KD):
                  nc.sync.dma_start(outT_d.ap()[dt * 128:(dt + 1) * 128, :], x1T[dt][:])
              xn2P.release(); x1pool.release(); cpool.release()
              nc.compile(); return nc

          # ---------------- phase D: FFN (shared + experts, fused) -------
          # hbuf holds all 96 h-tiles (scaled by 8 * routing weight); the down
          # projection accumulates rounds of 48 h-tiles straight into x1T.
          wD = tc.alloc_tile_pool(name=f"wD{rep_i}", bufs=1)
          hD = tc.alloc_tile_pool(name=f"hD{rep_i}", bufs=1)
          psD = tc.alloc_tile_pool(name=f"psD{rep_i}", bufs=1, space="PSUM")
          hbufs = [hD.tile([128, ROUND * S], f8, name=f"hbuf{r}", tag=f"hbuf{r}")
                   for r in range(2)]
          hb3s = [h[:].rearrange("p (i s) -> p i s", i=ROUND) for h in hbufs]

          def hb_slot(slot):
              return hb3s[slot // ROUND], slot % ROUND

          def gate_up(wg_ap, wg_nb, wg_b, wu_ap, wu_nb, wu_b, slot):
              """hbuf[slot] = 8 * rw * silu(xn2 @ wg) * (xn2 @ wu)"""
              wg = load_blk(wD, wg_ap, wg_nb, wg_b, KD * 128, tag="wg", bufs=3,
                            name="wg")
              wu = load_blk(wD, wu_ap, wu_nb, wu_b, KD * 128, tag="wu", bufs=3,
                            name="wu")
              wg3 = wg[:].rearrange("p (k m) -> p k m", k=KD)
              wu3 = wu[:].rearrange("p (k m) -> p k m", k=KD)
              pg2 = psD.tile([128, 2 * CH], f32, name="pg2", tag="ps", bufs=4)
              pu2 = psD.tile([128, 2 * CH], f32, name="pu2", tag="ps", bufs=4)
              for kp in range(KP):
                  for c in range(NCH):
                      nc.tensor.matmul(pg2[:, c * CH:(c + 1) * CH],
                                       wg3[:, 2 * kp:2 * kp + 2, :],
                                       xn23[:, 2 * kp:2 * kp + 2, c * CH:(c + 1) * CH],
                                       start=(kp == 0), stop=(kp == KP - 1),
                                       perf_mode=PM.DoubleRow)
              for kp in range(KP):
                  for c in range(NCH):
                      nc.tensor.matmul(pu2[:, c * CH:(c + 1) * CH],
                                       wu3[:, 2 * kp:2 * kp + 2, :],
                                       xn23[:, 2 * kp:2 * kp + 2, c * CH:(c + 1) * CH],
                                       start=(kp == 0), stop=(kp == KP - 1),
                                       perf_mode=PM.DoubleRow)
              g = slot // GRP
              hb, si = hb_slot(slot)
              sg = hD.tile([128, 2 * CH], f32r, name="sg", tag="sg", bufs=2)
              nc.scalar.activation(sg[:], pg2[:], AF.Silu,
                                   scale=2.0 ** (-(SX + SW)))
              nc.vector.scalar_tensor_tensor(hb[:, si, :], pu2[:],
                                             dsc[:, g:g + 1],
                                             sg[:], op0=OP.mult, op1=OP.mult)

          def down(r):
              """x1T += g2 * (hbuf[r*48:(r+1)*48] @ wd)"""
              for dt in range(KD):
                  wd = load_blk(wD, wd_d.ap(), KD * 2, dt * 2 + r, ROUND * 128,
                                tag="wd", bufs=2, name="wd")
                  wd3 = wd[:].rearrange("p (i m) -> p i m", i=ROUND)
                  pd2 = psD.tile([128, 2 * CH], f32, name="pd2", tag="ps", bufs=4)
                  for ip in range(ROUND // 2):
                      for c in range(NCH):
                          nc.tensor.matmul(pd2[:, c * CH:(c + 1) * CH],
                                           wd3[:, 2 * ip:2 * ip + 2, :],
                                           hb3s[r][:, 2 * ip:2 * ip + 2,
                                                   c * CH:(c + 1) * CH],
                                           start=(ip == 0), stop=(ip == ROUND // 2 - 1),
                                           perf_mode=PM.DoubleRow)
                  nc.vector.scalar_tensor_tensor(x1T[dt][:, :], pd2[:],
                                                 g2c[:, dt:dt + 1],
                                                 x1T[dt][:, :],
                                                 op0=OP.mult, op1=OP.add)
                  if r == 1:
                      nc.sync.dma_start(outT_d.ap()[dt * 128:(dt + 1) * 128, :],
                                        x1T[dt][:])

          GRP = 8
          ht_args = []
          for g in range(SHT // GRP):
              for i in range(GRP):
                  ht_args.append((shwg_d.ap(), SHT, g * GRP + i,
                                  shwu_d.ap(), SHT, g * GRP + i))
          for e in range(NE):
              for i in range(RHT):
                  ht_args.append((rwg_d.ap(), NE * RHT, e * RHT + i,
                                  rwu_d.ap(), NE * RHT, e * RHT + i))
          if SKIP_FFN:
              for dt in range(KD):
                  nc.sync.dma_start(outT_d.ap()[dt * 128:(dt + 1) * 128, :],
                                    x1T[dt][:])
          else:
              for slot, args in enumerate(ht_args):
                  gate_up(*args, slot)
                  if slot == ROUND - 1:
                      down(0)
              down(1)
          psD.release()
          hD.release()
          wD.release()
          xn2P.release()
          x1pool.release()
          cpool.release()

    nc.compile()
    return nc


# ---------------------------------------------------------------- entry point

_CACHE = {}

_IN_ORDER = ["src", "centroids", "routing_weights", "qkv_w", "qkv_b", "out_w",
             "out_b", "norm1_w", "norm2_w", "gamma_1", "gamma_2",
             "sh_wg", "sh_bg", "sh_wu", "sh_bu", "sh_wd", "sh_bd",
             "r_wg", "r_bg", "r_wu", "r_bu", "r_wd", "r_bd"]


def _prep(cfg, inputs):
    args = [np.asarray(inputs[k]) for k in _IN_ORDER]
    return host_prep(cfg, *args)


def kernel(**inputs):
    cfg = FULL
    in_maps = _prep(cfg, inputs)
    if "nc" not in _CACHE:
        _CACHE["nc"] = build_nc(cfg)
    nc = _CACHE["nc"]
    res = bass_utils.run_bass_kernel_spmd(nc, in_maps, core_ids=list(range(cfg["NCORES"])))
    B, S, D = np.asarray(inputs["src"]).shape
    out = np.empty((B, S, D), np.float32)
    for b in range(B):
        out[b] = res.results[b]["outT"].T
    return out
